# revision 7
# baseline (speedup 1.0000x reference)
"""Trainium2 Bass kernel for PseudoLabelPlus.

Math shortcut: the outputs (p, q_kplus) are [B,10] quantities that only
depend on per-token dot products emb[tok]@conv_w.T and emb[tok]@Wk, never
on raw embeddings. So instead of gathering [B,S,300] embeddings (315MB),
precompute the score table G = emb_table @ [conv_w.T | Wk] (+bias via an
augmented ones-row) of shape [50000, 19], vocab-sharded across 8 cores,
AllGather it (4MB), then gather 80-byte rows per token.

Per-batch reductions used (S = seq len = 128, all exact rewrites):
  w_ts   = relu(max_f (TS[v_ts,f] + conv_b_f))          (relu/max commute)
  A_bf   = sum_s w * (TS + conv_b_f)  -> sen@conv_w.T = (A - conv_b*sum_s w)/S
  K_bk   = sum_s (TK + bk_k)          -> mean@Wk + bk  = K/S
fs = sum_b q_kplus needs a tiny AllReduce before the final normalization.
"""

import os
import sys
import types

import numpy as np

from concourse import bacc, bass, mybir, tile
from concourse.bass import IndirectOffsetOnAxis
from concourse.bass_utils import run_bass_kernel_spmd


def _install_ntff_hook_shim():
    """Register the NTFF profiling hook bass_utils expects under axon.

    The agent image's ``antenv`` lacks ``axon_hooks``; the hook machinery
    itself lives in ``trn_agent_boot``. Harmless no-op when unavailable.
    """
    try:
        from antenv.axon_hooks import get_axon_ntff_profile_hook  # noqa: F401
        return
    except ImportError:
        pass
    hook = None
    try:
        from trn_agent_boot.trn_boot import _ntff_profile_via_ctypes
        so = "/opt/axon/libaxon_pjrt.so"
        if os.path.exists(so):
            hook = _ntff_profile_via_ctypes(so)
    except Exception:
        hook = None
    mod = types.ModuleType("antenv.axon_hooks")
    state = {"hook": hook}
    mod.get_axon_ntff_profile_hook = lambda: state["hook"]
    mod.set_axon_ntff_profile_hook = lambda h: state.__setitem__("hook", h)
    sys.modules["antenv.axon_hooks"] = mod
    try:
        import antenv
        antenv.axon_hooks = mod
    except ImportError:
        pass


_install_ntff_hook_shim()

M = 8            # cores
S = 128          # seq len
B = 2048         # batch
V = 50000        # vocab
E = 300          # emb dim
F = 9            # conv filters
K = 10           # classes
BC = B // M      # 256 batch rows per core
VC = V // M      # 6250 vocab rows per core
NW = F + K       # 19 real table columns
GW = 20          # padded table row width (80B rows)
EA = E + 1       # emb dim augmented with ones row (bias fold)
FDT = mybir.dt.float32
IDT = mybir.dt.int32
AX = mybir.AxisListType.X
OP = mybir.AluOpType
ACT = mybir.ActivationFunctionType

KCH = [(0, 128), (128, 256), (256, EA)]   # contraction chunks over EA=301
NVT = (VC + 127) // 128                   # 49 vocab tiles per core
LN10 = float(np.log(10.0))

_CACHE = {}


def _build_program():
    nc = bacc.Bacc(None, num_devices=M)

    emb_t = nc.dram_tensor("emb_t", [EA, VC], FDT, kind="ExternalInput")
    wmat = nc.dram_tensor("wmat", [EA, NW], FDT, kind="ExternalInput")
    cb = nc.dram_tensor("cb", [1, F], FDT, kind="ExternalInput")
    tok = nc.dram_tensor("tok", [BC, S], IDT, kind="ExternalInput")
    out_p = nc.dram_tensor("out_p", [BC, K], FDT, kind="ExternalOutput")
    out_q = nc.dram_tensor("out_q", [BC, K], FDT, kind="ExternalOutput")

    g_loc = nc.dram_tensor("g_loc", [VC, GW], FDT)
    g_full = nc.dram_tensor("g_full", [V, GW], FDT, addr_space="Shared")
    fs_loc = nc.dram_tensor("fs_loc", [1, 16], FDT)
    fs_glob = nc.dram_tensor("fs_glob", [1, 16], FDT, addr_space="Shared")

    rg = [list(range(M))]

    with tile.TileContext(nc) as tc:
        with (
            tc.tile_pool(name="const", bufs=1) as pc,
            tc.tile_pool(name="half", bufs=2) as ph,
            tc.tile_pool(name="mm", bufs=4, space="PSUM") as pmm,
            tc.tile_pool(name="ps1", bufs=1, space="PSUM") as ps1,
        ):
            # ---------------- Phase A: score table G ----------------
            emb_sb = []
            for c, (k0, k1) in enumerate(KCH):
                t = pc.tile([k1 - k0, VC], FDT, name=f"emb{c}")
                nc.sync.dma_start(out=t[:], in_=emb_t[k0:k1, :])
                emb_sb.append(t)
            wm_sb = []
            for c, (k0, k1) in enumerate(KCH):
                t = pc.tile([k1 - k0, NW], FDT, name=f"wm{c}")
                nc.sync.dma_start(out=t[:], in_=wmat[k0:k1, :])
                wm_sb.append(t)

            stage = pc.tile([128, NVT, GW], FDT, name="stage")
            nc.vector.memset(stage[:], 0.0)
            for t in range(NVT):
                v0 = t * 128
                sz = min(128, VC - v0)
                ps = pmm.tile([128, NW], FDT, name="mmout")
                for c, (k0, k1) in enumerate(KCH):
                    nc.tensor.matmul(
                        ps[:sz, :],
                        lhsT=emb_sb[c][:, v0:v0 + sz],
                        rhs=wm_sb[c][:],
                        start=(c == 0),
                        stop=(c == len(KCH) - 1),
                    )
                nc.vector.tensor_copy(stage[:sz, t, 0:NW], ps[:sz, :])

            nfull = (NVT - 1) * 128
            nc.sync.dma_start(
                out=g_loc[0:nfull, :].rearrange("(t p) f -> p t f", p=128),
                in_=stage[:, 0:NVT - 1, :],
            )
            nc.sync.dma_start(
                out=g_loc[nfull:VC, :], in_=stage[:VC - nfull, NVT - 1, :]
            )
            nc.gpsimd.collective_compute(
                "AllGather", OP.bypass, replica_groups=rg,
                ins=[g_loc[:].opt()], outs=[g_full[:].opt()],
            )

            # conv_b broadcast across partitions via K=1 matmul
            cb_sb = pc.tile([1, F], FDT, name="cb_sb")
            nc.sync.dma_start(out=cb_sb[:], in_=cb[:])
            ones_k1 = pc.tile([1, 128], FDT, name="ones_k1")
            nc.vector.memset(ones_k1[:], 1.0)
            ps_cb = ps1.tile([128, F], FDT, name="cbps")
            nc.tensor.matmul(ps_cb[:], lhsT=ones_k1[:], rhs=cb_sb[:],
                             start=True, stop=True)
            cb128 = pc.tile([128, F], FDT, name="cb128")
            nc.vector.tensor_copy(cb128[:], ps_cb[:])
            cb_b = cb128[:].unsqueeze(1).to_broadcast([128, 2, F])

            # ---------------- Phase B: gather + per-batch sums ----------------
            A2 = pc.tile([128, 2, F], FDT, name="A2")
            K2 = pc.tile([128, 2, K], FDT, name="K2")
            Ws2 = pc.tile([128, 2], FDT, name="Ws2")
            for h in range(2):
                idx = ph.tile([128, S], IDT, name="idx")
                nc.sync.dma_start(out=idx[:], in_=tok[h * 128:(h + 1) * 128, :])
                y = ph.tile([128, S, GW], FDT, name="y")
                nc.gpsimd.indirect_dma_start(
                    out=y[:], out_offset=None,
                    in_=g_full[:],
                    in_offset=IndirectOffsetOnAxis(ap=idx[:], axis=0),
                )
                wmx = ph.tile([128, S], FDT, name="wmx")
                nc.vector.tensor_reduce(out=wmx[:], in_=y[:, :, 0:F],
                                        axis=AX, op=OP.max)
                nc.vector.tensor_scalar_max(wmx[:], wmx[:], 0.0)
                wy = ph.tile([128, S, F], FDT, name="wy")
                nc.vector.tensor_tensor(
                    out=wy[:], in0=y[:, :, 0:F],
                    in1=wmx[:].unsqueeze(-1).to_broadcast([128, S, F]),
                    op=OP.mult,
                )
                nc.vector.tensor_reduce(
                    out=A2[:, h, :], in_=wy[:].rearrange("p s f -> p f s"),
                    axis=AX, op=OP.add,
                )
                nc.vector.tensor_reduce(
                    out=K2[:, h, :], in_=y[:, :, F:NW].rearrange("p s f -> p f s"),
                    axis=AX, op=OP.add,
                )
                nc.vector.tensor_reduce(out=Ws2[:, h:h + 1], in_=wmx[:],
                                        axis=AX, op=OP.add)

            # ---------------- Tail ----------------
            # q logits: relu((A2 - cb*Ws)/S + cb)
            t9 = pc.tile([128, 2, F], FDT, name="t9")
            nc.vector.tensor_tensor(
                out=t9[:], in0=Ws2[:].unsqueeze(-1).to_broadcast([128, 2, F]),
                in1=cb_b, op=OP.mult)
            nc.vector.tensor_sub(t9[:], A2[:], t9[:])
            nc.vector.tensor_scalar_mul(t9[:], t9[:], 1.0 / S)
            nc.vector.tensor_tensor(out=t9[:], in0=t9[:], in1=cb_b, op=OP.add)
            nc.vector.tensor_scalar_max(t9[:], t9[:], 0.0)

            # q = softmax(t9) over F
            m2 = pc.tile([128, 2], FDT, name="m2")
            nc.vector.tensor_reduce(out=m2[:], in_=t9[:], axis=AX, op=OP.max,
                                    negate=True)
            e9 = pc.tile([128, 2, F], FDT, name="e9")
            s2 = pc.tile([128, 2], FDT, name="s2")
            for h in range(2):
                nc.scalar.activation(out=e9[:, h, :], in_=t9[:, h, :],
                                     func=ACT.Exp, bias=m2[:, h:h + 1],
                                     scale=1.0, accum_out=s2[:, h:h + 1])
            r2 = pc.tile([128, 2], FDT, name="r2")
            nc.vector.reciprocal(r2[:], s2[:])
            q9 = pc.tile([128, 2, F], FDT, name="q9")
            nc.vector.tensor_tensor(
                out=q9[:], in0=e9[:],
                in1=r2[:].unsqueeze(-1).to_broadcast([128, 2, F]), op=OP.mult)

            # pred = softmax(K2/S) over K
            lk = pc.tile([128, 2, K], FDT, name="lk")
            nc.vector.tensor_scalar_mul(lk[:], K2[:], 1.0 / S)
            mk = pc.tile([128, 2], FDT, name="mk")
            nc.vector.tensor_reduce(out=mk[:], in_=lk[:], axis=AX, op=OP.max,
                                    negate=True)
            ek = pc.tile([128, 2, K], FDT, name="ek")
            sk = pc.tile([128, 2], FDT, name="sk")
            for h in range(2):
                nc.scalar.activation(out=ek[:, h, :], in_=lk[:, h, :],
                                     func=ACT.Exp, bias=mk[:, h:h + 1],
                                     scale=1.0, accum_out=sk[:, h:h + 1])
            rk = pc.tile([128, 2], FDT, name="rk")
            nc.vector.reciprocal(rk[:], sk[:])
            pk = pc.tile([128, 2, K], FDT, name="pk")
            nc.vector.tensor_tensor(
                out=pk[:], in0=ek[:],
                in1=rk[:].unsqueeze(-1).to_broadcast([128, 2, K]), op=OP.mult)

            # h_norm = -(sum pk*ln(pk))/ln(10); q_null = relu(2*h_norm - 1)
            lnp = pc.tile([128, 2, K], FDT, name="lnp")
            nc.scalar.activation(out=lnp[:], in_=pk[:], func=ACT.Ln)
            plp = pc.tile([128, 2, K], FDT, name="plp")
            nc.vector.tensor_tensor(out=plp[:], in0=pk[:], in1=lnp[:], op=OP.mult)
            hn = pc.tile([128, 2], FDT, name="hn")
            nc.vector.tensor_reduce(out=hn[:], in_=plp[:], axis=AX, op=OP.add)
            negone = pc.tile([128, 1], FDT, name="negone")
            nc.vector.memset(negone[:], -1.0)
            one_col = pc.tile([128, 1], FDT, name="one_col")
            nc.vector.memset(one_col[:], 1.0)
            qn = pc.tile([128, 2], FDT, name="qn")
            nc.scalar.activation(out=qn[:], in_=hn[:], func=ACT.Relu,
                                 bias=negone[:], scale=-2.0 / LN10)
            omq = pc.tile([128, 2], FDT, name="omq")
            nc.scalar.activation(out=omq[:], in_=qn[:], func=ACT.Identity,
                                 bias=one_col[:], scale=-1.0)

            # q_kplus: [q_k[0:4], q_null, q_k[4:9]]
            qk = pc.tile([128, 2, F], FDT, name="qk")
            nc.vector.tensor_tensor(
                out=qk[:], in0=q9[:],
                in1=omq[:].unsqueeze(-1).to_broadcast([128, 2, F]), op=OP.mult)
            QK = pc.tile([128, 2, K], FDT, name="QK")
            nc.vector.tensor_copy(QK[:, :, 0:4], qk[:, :, 0:4])
            nc.vector.tensor_copy(QK[:, :, 4:5], qn[:].unsqueeze(-1))
            nc.vector.tensor_copy(QK[:, :, 5:K], qk[:, :, 4:F])

            # fs partial: sum over the 128 partitions via ones-matmul
            ones = pc.tile([128, 1], FDT, name="ones")
            nc.vector.memset(ones[:], 1.0)
            ps_fs = ps1.tile([1, 2 * K], FDT, name="fsps")
            nc.tensor.matmul(ps_fs[:], lhsT=ones[:],
                             rhs=QK[:].rearrange("p h f -> p (h f)"),
                             start=True, stop=True)
            fs_hk = pc.tile([1, 2 * K], FDT, name="fs_hk")
            nc.vector.tensor_copy(fs_hk[:], ps_fs[:])
            fs_sb = pc.tile([1, 16], FDT, name="fs_sb")
            nc.vector.memset(fs_sb[:], 0.0)
            nc.vector.tensor_add(fs_sb[:, 0:K], fs_hk[:, 0:K], fs_hk[:, K:2 * K])
            nc.sync.dma_start(out=fs_loc[:], in_=fs_sb[:])
            nc.gpsimd.collective_compute(
                "AllReduce", OP.add, replica_groups=rg,
                ins=[fs_loc[:].opt()], outs=[fs_glob[:].opt()],
            )
            fsg = pc.tile([1, 16], FDT, name="fsg")
            nc.sync.dma_start(out=fsg[:], in_=fs_glob[:])
            rfs = pc.tile([1, K], FDT, name="rfs")
            nc.vector.reciprocal(rfs[:], fsg[:, 0:K])
            ps_rb = ps1.tile([128, K], FDT, name="rbps")
            nc.tensor.matmul(ps_rb[:], lhsT=ones_k1[:], rhs=rfs[:],
                             start=True, stop=True)
            rb = pc.tile([128, K], FDT, name="rb")
            nc.vector.tensor_copy(rb[:], ps_rb[:])

            # p = (QK^2/fs) normalized over K
            sq = pc.tile([128, 2, K], FDT, name="sq")
            nc.vector.tensor_tensor(out=sq[:], in0=QK[:], in1=QK[:], op=OP.mult)
            nc.vector.tensor_tensor(
                out=sq[:], in0=sq[:],
                in1=rb[:].unsqueeze(1).to_broadcast([128, 2, K]), op=OP.mult)
            rs = pc.tile([128, 2], FDT, name="rs")
            nc.vector.tensor_reduce(out=rs[:], in_=sq[:], axis=AX, op=OP.add)
            rr = pc.tile([128, 2], FDT, name="rr")
            nc.vector.reciprocal(rr[:], rs[:])
            pfin = pc.tile([128, 2, K], FDT, name="pfin")
            nc.vector.tensor_tensor(
                out=pfin[:], in0=sq[:],
                in1=rr[:].unsqueeze(-1).to_broadcast([128, 2, K]), op=OP.mult)

            nc.sync.dma_start(
                out=out_p[:].rearrange("(h p) f -> p h f", h=2), in_=pfin[:])
            nc.sync.dma_start(
                out=out_q[:].rearrange("(h p) f -> p h f", h=2), in_=QK[:])

    nc.finalize()
    return nc


def _prep_in_maps(text, emb_table, conv_w, conv_b, Wk, bk):
    tokT = np.ascontiguousarray(np.asarray(text).astype(np.int32).T)  # [B, S]
    embf = np.asarray(emb_table, dtype=np.float32)
    conv_w = np.asarray(conv_w, dtype=np.float32)
    conv_b = np.asarray(conv_b, dtype=np.float32)
    Wk = np.asarray(Wk, dtype=np.float32)
    bk = np.asarray(bk, dtype=np.float32)

    W = np.concatenate([conv_w.T, Wk], axis=1)               # [E, NW]
    bias_row = np.concatenate([conv_b, bk])[None, :]          # [1, NW]
    waug = np.ascontiguousarray(np.concatenate([W, bias_row], axis=0))
    cbm = np.ascontiguousarray(conv_b.reshape(1, F))

    in_maps = []
    for i in range(M):
        esl = embf[i * VC:(i + 1) * VC]
        embT = np.empty((EA, VC), dtype=np.float32)
        embT[:E] = esl.T
        embT[E] = 1.0
        in_maps.append({
            "emb_t": embT,
            "wmat": waug,
            "cb": cbm,
            "tok": np.ascontiguousarray(tokT[i * BC:(i + 1) * BC]),
        })
    return in_maps


TRACE = os.environ.get("KERNEL_TRACE", "0") == "1"
LAST = {}


def kernel(text, emb_table, conv_w, conv_b, Wk, bk):
    if "nc" not in _CACHE:
        _CACHE["nc"] = _build_program()
    nc = _CACHE["nc"]
    in_maps = _prep_in_maps(text, emb_table, conv_w, conv_b, Wk, bk)

    res = run_bass_kernel_spmd(
        nc, in_maps, core_ids=list(range(M)), trace=TRACE,
    )
    LAST["res"] = res
    p = np.concatenate([r["out_p"] for r in res.results], axis=0)
    q = np.concatenate([r["out_q"] for r in res.results], axis=0)
    return p, q


def _run_sim(text, emb_table, conv_w, conv_b, Wk, bk):
    """CoreSim-based check (no hardware)."""
    from concourse import bass_interp
    if "nc" not in _CACHE:
        _CACHE["nc"] = _build_program()
    nc = _CACHE["nc"]
    in_maps = _prep_in_maps(text, emb_table, conv_w, conv_b, Wk, bk)
    sim = bass_interp.MultiCoreSim(nc, M)
    for i in range(M):
        for k, v in in_maps[i].items():
            sim.cores[i].tensor(k)[:] = v
    sim.simulate()
    p = np.concatenate([np.asarray(sim.cores[i].mem_tensor("out_p"))
                        for i in range(M)], axis=0)
    q = np.concatenate([np.asarray(sim.cores[i].mem_tensor("out_q"))
                        for i in range(M)], axis=0)
    return p, q
